# revision 2
# baseline (speedup 1.0000x reference)
"""Trainium2 Bass kernel v5: entmax-1.5 along last dim of x[8,16,1024,1024] f32.

Row-parallel over 8 NeuronCores. Single x read; y computed from SBUF-resident
f16 h, no second DRAM read (HBM traffic = 1 read + 1 write = the floor).

Algorithm per row (d=1024), h = f16(x/2), tau in h-space:
  seed: closed-form k=8 threshold over the 8 chunk maxima of h.
  2 iterations of the estimated-support quadratic update (K-free):
    r  = max(h, t16)          DVE tensor_scalar + accum -> A1 (s1 = A1-1024t)
    s2 = sum (r - t16)^2      ACT Square(bias=-t16) + accum
    k_hat = s1^2/s2  =>  theta = (s2 - sqrt(s2))/s1, clamped at -2
    (exact when support gaps are equal; converges faster than plain Newton,
     needs no is_gt count - validated 2.7e-3 in numpy vs f64 oracle)
  final: y = (max(h, f16(t2)) - t2)^2 via ACT Square with f32 bias -t2;
    quantizing the FINAL threshold makes the off-support floor ~ulp^2.

HW lessons (measured on these cores, cost model underestimates):
  - DVE accum_out streaming ops ~0.83us ([P,1024] f16), is_gt+accum 1.19us,
    plain 0.43us; ACT Square ~0.82-0.92us regardless of bias/accum.
  - tensor_scalar with accum_out: op1 is the REDUCTION op, not a second
    scalar op (hence r-form + chain-side s1 correction).
  - tensor_tensor_reduce crashes the runtime (NRT_EXEC_UNIT_UNRECOVERABLE).
  - Engine balance per tile: DVE ~2.9us, ACT ~2.8us, DMA 2.9us - all at
    the 358GB/s/core memory roofline.
"""

import sys

sys.path.insert(0, "/opt/trn_rl_repo")
sys.path.insert(0, "/opt/trn_rl_repo/concourse")

from contextlib import ExitStack

import numpy as np

D = 1024
P = 128
N_CORES = 8


def build_program(n_rows, G=8, B=4, span=2, iters=2, fresh_k=True,
                  newton=False,
                  newton2=False, cast_act=False, s2_modes=("act", "act"),
                  s2_dve=(0, 0), persist_z=True, sqrt_act=True,
                  lookahead=0, weave=1, debug=False, reps=1, dump=False,
                  xp_bufs=2, hp_bufs=5, zp_bufs=7, scr_bufs=4, yp_bufs=2,
                  ablate=()):
    import concourse.bacc as bacc
    import concourse.tile as tile
    from concourse import mybir

    F32 = mybir.dt.float32
    F16 = mybir.dt.float16
    ALU = mybir.AluOpType
    ACTF = mybir.ActivationFunctionType
    AX = mybir.AxisListType

    nb = G // B
    T = n_rows // P
    assert n_rows % P == 0 and T % (G * span) == 0 and G % B == 0
    n_spans = T // (G * span)
    W = G * span
    tiles_per_span = span * nb  # number of [P,B,D] tiles

    nc = bacc.Bacc(
        "TRN2", target_bir_lowering=False, debug=debug, enable_asserts=False
    )
    x = nc.dram_tensor("x", [n_rows, D], F32, kind="ExternalInput").ap()
    y = nc.dram_tensor("y", [n_rows, D], F32, kind="ExternalOutput").ap()
    dmp = None
    if dump:
        # per span-state dump: [P, n_spans, 8, W]: t0,K0,A1_0,S2_0,t1,K1,A1_1,S2_1
        n_spans_ = (n_rows // P) // (G * span)
        dmp = nc.dram_tensor(
            "dmp", [P, n_spans_ * 8 * G * span], F32, kind="ExternalOutput"
        ).ap()

    with tile.TileContext(nc) as tc, ExitStack() as ctx:
        xp = ctx.enter_context(tc.tile_pool(name="xp", bufs=xp_bufs))
        hp = ctx.enter_context(tc.tile_pool(name="hp", bufs=hp_bufs))
        zp = ctx.enter_context(tc.tile_pool(name="zp", bufs=zp_bufs))
        scr = ctx.enter_context(tc.tile_pool(name="scr", bufs=scr_bufs))
        c8p = ctx.enter_context(tc.tile_pool(name="c8p", bufs=2))
        yp = ctx.enter_context(tc.tile_pool(name="yp", bufs=yp_bufs))
        smp = ctx.enter_context(tc.tile_pool(name="smp", bufs=2))

        src = {"t": x}

        def stage_load_cast_seed(s0):
            """s0: first group index of the span. Returns pair-state."""
            ps = {"hb": [], "s0": s0}
            stage_load_bulk(ps)
            stage_seed_chain(ps)
            return ps

        def stage_load_bulk(ps):
            s0 = ps["s0"]
            c8 = c8p.tile([P, W, 8], F32, tag="c8")
            ps["c8"] = c8
            for gi in range(span):
                g_row0 = (s0 + gi) * G * P
                for b in range(nb):
                    r0 = g_row0 + b * B * P
                    xt = xp.tile([P, B, D], F32, tag="x")
                    if "no_dma_in" not in ablate:
                        nc.sync.dma_start(
                            xt[:],
                            src["t"][r0 : r0 + B * P, :].rearrange(
                                "(a p) m -> p a m", p=P
                            ),
                        )
                    else:
                        nc.vector.memset(xt[:, 0:1, 0:8], 0.5)
                    ht = hp.tile([P, B, D], F16, tag="h")
                    if cast_act:
                        nc.scalar.activation(
                            ht[:], xt[:], ACTF.Copy, scale=0.5
                        )
                    else:
                        nc.vector.tensor_scalar(
                            ht[:], xt[:], 0.5, None, op0=ALU.mult
                        )
                    ps["hb"].append(ht)
                    if "no_seed" not in ablate:
                        for jj in range(B):
                            q = gi * G + b * B + jj
                            nc.vector.tensor_reduce(
                                c8[:, q, :],
                                ht[:, jj, :].rearrange(
                                    "p (c e) -> p c e", e=128
                                ),
                                axis=AX.X,
                                op=ALU.max,
                            )

        def stage_seed_chain(ps):
            c8 = ps["c8"]
            if "no_seed" in ablate:
                t0 = smp.tile([P, W], F32, tag="t0")
                nc.vector.memset(t0[:], 1.0)
                ps["t"] = t0
                return ps

            # merged seed chain (k=8 closed form over chunk maxima)
            m = smp.tile([P, W], F32, tag="m")
            nc.vector.tensor_reduce(m[:], c8[:], axis=AX.X, op=ALU.max)
            S = smp.tile([P, W], F32, tag="S")
            nc.vector.tensor_reduce(S[:], c8[:], axis=AX.X, op=ALU.add)
            c8sq = c8p.tile([P, W, 8], F32, tag="c8sq")
            nc.vector.tensor_tensor(c8sq[:], c8[:], c8[:], op=ALU.mult)
            Q = smp.tile([P, W], F32, tag="Q")
            nc.vector.tensor_reduce(Q[:], c8sq[:], axis=AX.X, op=ALU.add)
            mm = smp.tile([P, W], F32, tag="mm")
            nc.vector.tensor_scalar_mul(mm[:], m[:], -8.0)
            s1 = smp.tile([P, W], F32, tag="s1")
            nc.vector.tensor_tensor(s1[:], S[:], mm[:], op=ALU.add)
            mS = smp.tile([P, W], F32, tag="mS")
            nc.vector.tensor_tensor(mS[:], m[:], S[:], op=ALU.mult)
            m2 = smp.tile([P, W], F32, tag="m2")
            nc.vector.tensor_tensor(m2[:], m[:], m[:], op=ALU.mult)
            a1 = smp.tile([P, W], F32, tag="a1")
            nc.vector.tensor_scalar_mul(a1[:], mS[:], -2.0)
            a2 = smp.tile([P, W], F32, tag="a2")
            nc.vector.tensor_scalar_mul(a2[:], m2[:], 8.0)
            s2a = smp.tile([P, W], F32, tag="s2a")
            nc.vector.tensor_tensor(s2a[:], Q[:], a1[:], op=ALU.add)
            s2 = smp.tile([P, W], F32, tag="s2")
            nc.vector.tensor_tensor(s2[:], s2a[:], a2[:], op=ALU.add)
            q_ = smp.tile([P, W], F32, tag="q")
            nc.vector.tensor_tensor(q_[:], s1[:], s1[:], op=ALU.mult)
            b1 = smp.tile([P, W], F32, tag="b1")
            nc.vector.tensor_scalar(
                b1[:], s2[:], -8.0, 8.0, op0=ALU.mult, op1=ALU.add
            )
            d0 = smp.tile([P, W], F32, tag="d0")
            nc.vector.tensor_tensor(d0[:], q_[:], b1[:], op=ALU.add)
            dn = smp.tile([P, W], F32, tag="dn")
            nc.vector.tensor_scalar_max(dn[:], d0[:], 1e-30)
            root = smp.tile([P, W], F32, tag="root")
            nc.scalar.activation(root[:], dn[:], ACTF.Sqrt)
            num = smp.tile([P, W], F32, tag="num")
            nc.vector.tensor_tensor(num[:], s1[:], root[:], op=ALU.subtract)
            th = smp.tile([P, W], F32, tag="th")
            nc.vector.tensor_scalar_mul(th[:], num[:], 0.125)
            t0 = smp.tile([P, W], F32, tag="t0")
            nc.vector.tensor_tensor(t0[:], m[:], th[:], op=ALU.add)
            ps["t"] = t0
            return ps

        def q16(ps):
            t16h = smp.tile([P, W], F16, tag="t16h")
            nc.vector.tensor_scalar_mul(t16h[:], ps["t"][:], 1.0)
            t16f = smp.tile([P, W], F32, tag="t16f")
            nc.vector.tensor_scalar_mul(t16f[:], t16h[:], 1.0)
            tb = smp.tile([P, W], F32, tag="tb")
            nc.vector.tensor_scalar_mul(tb[:], t16f[:], -1.0)
            return t16f, tb

        def solve(ps, t16f, K, A1, S2):
            """exact fixed-support solve -> new t (f32).

            A1 = sum(max(h,t16)) so s1 = A1 - 1024*t16."""
            tm = smp.tile([P, W], F32, tag="tm")
            nc.vector.tensor_scalar_mul(tm[:], t16f[:], -1024.0)
            s1v = smp.tile([P, W], F32, tag="s1v")
            nc.vector.tensor_tensor(s1v[:], A1[:], tm[:], op=ALU.add)
            s1g = smp.tile([P, W], F32, tag="s1g")
            nc.vector.tensor_scalar_max(s1g[:], s1v[:], 1e-6)
            if newton == "estk":
                # k_hat = s1^2/s2 exact-quadratic: th = (s2 - sqrt(s2))/s1
                rp = smp.tile([P, W], F32, tag="rpn")
                nc.vector.reciprocal(rp[:], s1g[:])
                s2g = smp.tile([P, W], F32, tag="s2g")
                nc.vector.tensor_scalar_max(s2g[:], S2[:], 0.0)
                rt = smp.tile([P, W], F32, tag="rtn")
                nc.scalar.activation(rt[:], s2g[:], ACTF.Sqrt)
                g1 = smp.tile([P, W], F32, tag="g1n")
                nc.vector.tensor_tensor(
                    g1[:], s2g[:], rt[:], op=ALU.subtract
                )
                th1 = smp.tile([P, W], F32, tag="th1n")
                nc.vector.tensor_tensor(th1[:], g1[:], rp[:], op=ALU.mult)
                thc = smp.tile([P, W], F32, tag="thcn")
                nc.vector.tensor_scalar_max(thc[:], th1[:], -2.0)
                t_new = smp.tile([P, W], F32, tag="tn")
                nc.vector.tensor_tensor(
                    t_new[:], t16f[:], thc[:], op=ALU.add
                )
                return t_new
            if newton:
                # Newton step on f(t)=sum((h-t)_+^2)-1: th = (s2-1)/(2 s1)
                den = smp.tile([P, W], F32, tag="den")
                nc.vector.tensor_scalar_mul(den[:], s1g[:], 2.0)
                rp = smp.tile([P, W], F32, tag="rpn")
                nc.vector.reciprocal(rp[:], den[:])
                g1 = smp.tile([P, W], F32, tag="g1n")
                nc.vector.tensor_scalar(
                    g1[:], S2[:], -1.0, None, op0=ALU.add
                )
                th1 = smp.tile([P, W], F32, tag="th1n")
                nc.vector.tensor_tensor(th1[:], g1[:], rp[:], op=ALU.mult)
                thc = smp.tile([P, W], F32, tag="thcn")
                nc.vector.tensor_scalar_max(thc[:], th1[:], -2.0)
                t_new = smp.tile([P, W], F32, tag="tn")
                nc.vector.tensor_tensor(
                    t_new[:], t16f[:], thc[:], op=ALU.add
                )
                return t_new
            kg = smp.tile([P, W], F32, tag="kg")
            nc.vector.tensor_scalar_max(kg[:], K[:], 1.0)
            if not newton2:
                g1 = smp.tile([P, W], F32, tag="g1")
                nc.vector.tensor_scalar(
                    g1[:], S2[:], -1.0, None, op0=ALU.add
                )
                e = smp.tile([P, W], F32, tag="e")
                nc.vector.tensor_tensor(e[:], kg[:], g1[:], op=ALU.mult)
                p1 = smp.tile([P, W], F32, tag="p1")
                nc.vector.tensor_tensor(p1[:], s1g[:], s1g[:], op=ALU.mult)
                disc = smp.tile([P, W], F32, tag="disc")
                nc.vector.tensor_tensor(
                    disc[:], p1[:], e[:], op=ALU.subtract
                )
                dn2 = smp.tile([P, W], F32, tag="dn2")
                nc.vector.tensor_scalar_max(dn2[:], disc[:], 0.0)
                if sqrt_act:
                    root = smp.tile([P, W], F32, tag="root2")
                    nc.scalar.activation(root[:], dn2[:], ACTF.Sqrt)
                else:
                    raise NotImplementedError
                num = smp.tile([P, W], F32, tag="num2")
                nc.vector.tensor_tensor(
                    num[:], s1g[:], root[:], op=ALU.subtract
                )
                rk = smp.tile([P, W], F32, tag="rk")
                nc.vector.reciprocal(rk[:], kg[:])
                th = smp.tile([P, W], F32, tag="th2")
                nc.vector.tensor_tensor(th[:], num[:], rk[:], op=ALU.mult)
                thc = smp.tile([P, W], F32, tag="thc")
                nc.vector.tensor_scalar_max(thc[:], th[:], -2.0)
            else:
                # baseline's all-DVE 2-step-Newton on the quadratic
                g1 = smp.tile([P, W], F32, tag="g1")
                nc.vector.tensor_scalar(
                    g1[:], S2[:], -1.0, None, op0=ALU.add
                )
                rp = smp.tile([P, W], F32, tag="rp")
                nc.vector.reciprocal(rp[:], s1g[:])
                a_ = smp.tile([P, W], F32, tag="a_")
                nc.vector.tensor_tensor(a_[:], g1[:], rp[:], op=ALU.mult)
                th1 = smp.tile([P, W], F32, tag="th1")
                nc.vector.tensor_scalar_mul(th1[:], a_[:], 0.5)
                e = smp.tile([P, W], F32, tag="e")
                nc.vector.tensor_tensor(e[:], kg[:], th1[:], op=ALU.mult)
                c_ = smp.tile([P, W], F32, tag="c_")
                nc.vector.tensor_tensor(c_[:], e[:], s1g[:], op=ALU.subtract)
                c2 = smp.tile([P, W], F32, tag="c2")
                nc.vector.tensor_tensor(c2[:], c_[:], s1g[:], op=ALU.subtract)
                u_ = smp.tile([P, W], F32, tag="u_")
                nc.vector.tensor_tensor(u_[:], th1[:], c2[:], op=ALU.mult)
                qv = smp.tile([P, W], F32, tag="qv")
                nc.vector.tensor_tensor(qv[:], u_[:], g1[:], op=ALU.add)
                qp = smp.tile([P, W], F32, tag="qp")
                nc.vector.tensor_scalar_mul(qp[:], c_[:], 2.0)
                rq = smp.tile([P, W], F32, tag="rq")
                nc.vector.reciprocal(rq[:], qp[:])
                d_ = smp.tile([P, W], F32, tag="d_")
                nc.vector.tensor_tensor(d_[:], qv[:], rq[:], op=ALU.mult)
                th = smp.tile([P, W], F32, tag="th2")
                nc.vector.tensor_tensor(th[:], th1[:], d_[:], op=ALU.subtract)
                rk = smp.tile([P, W], F32, tag="rk")
                nc.vector.reciprocal(rk[:], kg[:])
                thv = smp.tile([P, W], F32, tag="thv")
                nc.vector.tensor_tensor(thv[:], s1g[:], rk[:], op=ALU.mult)
                thm = smp.tile([P, W], F32, tag="thm")
                nc.vector.tensor_tensor(thm[:], th[:], thv[:], op=ALU.min)
                thc = smp.tile([P, W], F32, tag="thc")
                nc.vector.tensor_scalar_max(thc[:], thm[:], -2.0)
            t_new = smp.tile([P, W], F32, tag="tn")
            nc.vector.tensor_tensor(t_new[:], t16f[:], thc[:], op=ALU.add)
            return t_new

        def stage_iter_pass(ps, it):
            last = it == iters - 1
            t16f, tb = q16(ps)
            s2_mode = s2_modes[it] if it < len(s2_modes) else s2_modes[-1]
            A1 = smp.tile([P, W], F32, tag="A1")
            S2 = smp.tile([P, W], F32, tag="S2")
            if "no_z" in ablate:
                nc.vector.memset(A1[:], 8.0)
            if "no_s2" in ablate:
                nc.vector.memset(S2[:], 1.5)
            if newton:
                K = None
                do_k = False
            elif it == 0 or fresh_k:
                K = smp.tile([P, W], F32, tag="K")
                ps["K"] = K
                do_k = True
            else:
                K = ps["K"]
                do_k = False
            if do_k and "no_k" in ablate:
                nc.vector.memset(K[:], 16.0)
            if last:
                ps["zb"] = []
                ps["t16f_last"] = t16f
            for gi in range(span):
                for b in range(nb):
                    ti = gi * nb + b
                    h_t = ps["hb"][ti]
                    if last and persist_z:
                        zt = zp.tile([P, B, D], F16, tag="z")
                        ps["zb"].append(zt)
                    for jj in range(B):
                        q = gi * G + b * B + jj
                        h_q = h_t[:, jj, :]
                        t_col = t16f[:, q : q + 1]
                        if last and persist_z:
                            r_q = zt[:, jj, :]
                        else:
                            z0 = scr.tile([P, D], F16, tag="z0")
                            r_q = z0[:]
                        # r = max(h, t16); accum A1 = sum(r)
                        if "no_z" not in ablate:
                            nc.vector.tensor_scalar(
                                r_q, h_q, t_col, None,
                                op0=ALU.max, op1=ALU.add,
                                accum_out=A1[:, q : q + 1],
                            )
                        if do_k and "no_k" not in ablate:
                            junk = scr.tile([P, D], F16, tag="junk")
                            nc.vector.tensor_scalar(
                                junk[:], h_q, t_col, None,
                                op0=ALU.is_gt, op1=ALU.add,
                                accum_out=K[:, q : q + 1],
                            )
                        if "no_s2" in ablate:
                            continue
                        s2col = S2[:, q : q + 1]
                        sqs = scr.tile([P, D], F16, tag="sq")
                        if s2_mode == "act":
                            # S2 = sum((r - t16)^2) via ACT Square w/ bias
                            nc.scalar.activation(
                                sqs[:], r_q, ACTF.Square, scale=1.0,
                                bias=tb[:, q : q + 1],
                                accum_out=s2col,
                            )
                        elif s2_mode == "act_nobias":
                            # r is biased on DVE first; plain ACT square
                            zb = scr.tile([P, D], F16, tag="zb")
                            nc.vector.tensor_scalar(
                                zb[:], r_q, tb[:, q : q + 1], None,
                                op0=ALU.add,
                            )
                            nc.scalar.activation(
                                sqs[:], zb[:], ACTF.Square, scale=1.0,
                                accum_out=s2col,
                            )
                        elif s2_mode == "ttr":
                            # R2 = sum(r^2); chain converts to s2.
                            # f32 out so the accumulated squares don't round
                            # through f16 (cancellation vs 1024*t^2).
                            sqf = scr.tile([P, D], F32, tag="sqf32")
                            nc.vector.tensor_tensor_reduce(
                                sqf[:], r_q, r_q, 1.0, 0.0,
                                ALU.mult, ALU.add, accum_out=s2col,
                            )
                        else:
                            raise ValueError(s2_mode)
            ps["iter_state"] = (t16f, K, A1, S2, s2_mode)

        def stage_iter_solve(ps, it):
            t16f, K, A1, S2, s2_mode = ps["iter_state"]
            if "no_solve" in ablate:
                return
            if s2_mode == "ttr" and "no_s2" not in ablate:
                # S2 holds R2 = sum(r^2); s2 = R2 - 2*t*A1 + 1024*t^2
                w1 = smp.tile([P, W], F32, tag="w1")
                nc.vector.tensor_scalar_mul(w1[:], A1[:], -2.0)
                w2 = smp.tile([P, W], F32, tag="w2")
                nc.vector.tensor_scalar_mul(w2[:], t16f[:], 1024.0)
                w3 = smp.tile([P, W], F32, tag="w3")
                nc.vector.tensor_tensor(w3[:], w1[:], w2[:], op=ALU.add)
                w4 = smp.tile([P, W], F32, tag="w4")
                nc.vector.tensor_tensor(w4[:], w3[:], t16f[:], op=ALU.mult)
                s2v = smp.tile([P, W], F32, tag="s2v")
                nc.vector.tensor_tensor(s2v[:], S2[:], w4[:], op=ALU.add)
                S2 = s2v
            ps["t"] = solve(ps, t16f, K, A1, S2)
            if dump:
                sp_i = ps["s0"] // span
                base = (sp_i * 8 + it * 4) * W
                for off, tile_ in ((0, t16f), (1, K), (2, A1), (3, S2)):
                    nc.sync.dma_start(
                        dmp[:, base + off * W : base + (off + 1) * W],
                        tile_[:],
                    )

        def stage_final(ps):
            if persist_z:
                t16f = ps["t16f_last"]
            else:
                # quantize the FINAL threshold: off-support y floor drops
                # from (t16_it1 - t2)^2 to ~ulp^2
                t2h = smp.tile([P, W], F16, tag="t2h")
                nc.vector.tensor_scalar_mul(t2h[:], ps["t"][:], 1.0)
                t16f = smp.tile([P, W], F32, tag="t2f")
                nc.vector.tensor_scalar_mul(t16f[:], t2h[:], 1.0)
            # y = (r_last - t_final)^2 with full f32 threshold precision
            bias = smp.tile([P, W], F32, tag="bias")
            nc.vector.tensor_scalar_mul(bias[:], ps["t"][:], -1.0)
            s0 = ps["s0"]
            for gi in range(span):
                g_row0 = (s0 + gi) * G * P
                for b in range(nb):
                    r0 = g_row0 + b * B * P
                    ti = gi * nb + b
                    yt = yp.tile([P, B, D], F32, tag="y")
                    for jj in range(B):
                        q = gi * G + b * B + jj
                        if persist_z:
                            r_q = ps["zb"][ti][:, jj, :]
                        else:
                            # recompute r from h vs the last t16
                            zf = scr.tile([P, D], F16, tag="zf")
                            nc.vector.tensor_scalar(
                                zf[:], ps["hb"][ti][:, jj, :],
                                t16f[:, q : q + 1], None,
                                op0=ALU.max,
                            )
                            r_q = zf[:]
                        nc.scalar.activation(
                            yt[:, jj, :], r_q, ACTF.Square,
                            scale=1.0, bias=bias[:, q : q + 1],
                        )
                    if "no_dma_out" not in ablate:
                        nc.sync.dma_start(
                            y[r0 : r0 + B * P, :].rearrange(
                                "(a p) m -> p a m", p=P
                            ),
                            yt[:],
                        )

        for rep in range(reps):
            if rep == 1:
                src["t"] = y
            if weave <= 1:
                if lookahead == 0:
                    for sp in range(n_spans):
                        ps = stage_load_cast_seed(sp * span)
                        for it in range(iters):
                            stage_iter_pass(ps, it)
                            stage_iter_solve(ps, it)
                        stage_final(ps)
                else:
                    pending = []
                    for sp in range(min(lookahead, n_spans)):
                        pending.append(stage_load_cast_seed(sp * span))
                    for sp in range(n_spans):
                        ps = pending.pop(0)
                        nxt = sp + lookahead
                        if nxt < n_spans:
                            pending.append(stage_load_cast_seed(nxt * span))
                        for it in range(iters):
                            stage_iter_pass(ps, it)
                            stage_iter_solve(ps, it)
                        stage_final(ps)
            else:
                assert n_spans % weave == 0
                for g0 in range(0, n_spans, weave):
                    group = [
                        stage_load_cast_seed((g0 + i) * span)
                        for i in range(weave)
                    ]
                    for it in range(iters):
                        for ps in group:
                            stage_iter_pass(ps, it)
                        for ps in group:
                            stage_iter_solve(ps, it)
                    for ps in group:
                        stage_final(ps)

    nc.compile()
    return nc


BEST_CFG = dict(
    newton="estk",
    iters=2,
    persist_z=False,
    zp_bufs=1,
    scr_bufs=5,
    hp_bufs=10,
    lookahead=1,
)

_PROGRAM = None
_PROGRAM_ROWS = None


def _get_program(rows_per_core):
    global _PROGRAM, _PROGRAM_ROWS
    if _PROGRAM is None or _PROGRAM_ROWS != rows_per_core:
        _PROGRAM = build_program(rows_per_core, **BEST_CFG)
        _PROGRAM_ROWS = rows_per_core
    return _PROGRAM


def build_timing_program(rows_per_core, reps=1):
    return build_program(rows_per_core, **BEST_CFG, reps=reps)


def run_sharded(flat_x, trace=False):
    from concourse.bass_utils import run_bass_kernel_spmd

    n_rows = flat_x.shape[0]
    rows_per = n_rows // N_CORES
    assert rows_per * N_CORES == n_rows
    nc = _get_program(rows_per)
    in_maps = [
        {"x": np.ascontiguousarray(flat_x[i * rows_per : (i + 1) * rows_per])}
        for i in range(N_CORES)
    ]
    res = run_bass_kernel_spmd(nc, in_maps, list(range(N_CORES)), trace=trace)
    y = np.concatenate([res.results[i]["y"] for i in range(N_CORES)], axis=0)
    return y, res


def kernel(x):
    x = np.ascontiguousarray(np.asarray(x), dtype=np.float32)
    orig_shape = x.shape
    flat = x.reshape(-1, D)
    y, _ = run_sharded(flat)
    return y.reshape(orig_shape)


if __name__ == "__main__":
    from concourse.timeline_sim import TimelineSim

    nc = build_program(16384)
    print("TimelineSim:", TimelineSim(nc, trace=False).simulate(), "ns")
